# revision 4
# baseline (speedup 1.0000x reference)
"""Trainium2 Bass kernel for nn_BaselineGNN (GNN message passing).

Strategy (8 NeuronCores, SPMD):
  - Node-partition the graph: core c owns dst nodes [c*12500, (c+1)*12500).
  - Edges live on the core that owns their dst; self-loops are appended as
    real edges.  Per core, edges are grouped by 128-node dst block and padded
    to 128-edge chunks.
  - Phase 0: global-context encoder u' = relu(gf@Wg+bg)@Wc2+bc  (tiny, every
    core computes it redundantly).
  - Phase 1 (own slice): h0 = relu(x@Wc1 + u'[batch]); m = relu(h0@Wm+bm)
    stored fp16; h0^T kept in SBUF for the update step.
  - AllGather m (fp16) so every core can gather any src row.
  - Phase 2: per dst block, aggr^T accumulates in PSUM as
    sum_chunks  m_gathered[128e,128h]^T @ onehot(dst)[128e,128d]
    followed by Ws^T@h0^T in the same PSUM group; then
    h^T = relu(aggr^T + bias), out^T = Wo^T@h^T + bo.
"""
import contextlib
import ctypes
import os
import sys
import types

sys.path.insert(0, "/opt/trn_rl_repo")

import numpy as np

import concourse.bass as bass
import concourse.bacc as bacc
import concourse.tile as tile
from concourse import mybir
from concourse.masks import make_identity

N_NODES, N_EDGES, N_GRAPHS = 100000, 1600000, 1024
IN_LOCAL, IN_GLOBAL, HIDDEN, NUM_CLASSES = 16, 8, 128, 2
P = 128
N_CORES = 8
SLICE = N_NODES // N_CORES            # 12500
NBLK = -(-SLICE // P)                 # 98
PAD_SLICE = NBLK * P                  # 12544
GBLK = N_GRAPHS // P                  # 8

f32 = mybir.dt.float32
f16 = mybir.dt.float16
i32 = mybir.dt.int32

_prog_cache: dict = {}
last_run: dict = {}


# --------------------------------------------------------------------------
# device program
# --------------------------------------------------------------------------
def _build(k_chunks: int):
    nchunks = NBLK * k_chunks
    nc = bacc.Bacc("TRN2", target_bir_lowering=False, debug=False,
                   num_devices=N_CORES)

    def inp(name, shape, dt):
        return nc.dram_tensor(name, shape, dt, kind="ExternalInput").ap()

    xT_d = inp("xT", [IN_LOCAL, PAD_SLICE], f32)
    gfT_d = inp("gfT", [IN_GLOBAL, N_GRAPHS], f32)
    ubi_d = inp("ubi", [P, NBLK], i32)
    srcT_d = inp("srcT", [P, nchunks], i32)
    dstT_d = inp("dstT", [P, nchunks], f16)
    iota_d = inp("iota", [P, k_chunks * P], f16)
    Wg_d = inp("Wg", [IN_GLOBAL, HIDDEN], f32)
    Wc1_d = inp("Wc1", [IN_LOCAL, HIDDEN], f32)
    Wc2_d = inp("Wc2", [HIDDEN, HIDDEN], f32)
    Wm_d = inp("Wm", [HIDDEN, HIDDEN], f32)
    Ws_d = inp("Ws", [HIDDEN, HIDDEN], f32)
    Wo_d = inp("Wo", [HIDDEN, NUM_CLASSES], f32)
    bg_d = inp("bg_c", [HIDDEN, 1], f32)
    bc_d = inp("bc_b", [P, HIDDEN], f32)
    bm_d = inp("bm_b", [P, HIDDEN], f32)
    bs_d = inp("bs_c", [HIDDEN, 1], f32)
    bo_d = inp("bo_c", [NUM_CLASSES, 1], f32)
    outT_d = nc.dram_tensor("outT", [NUM_CLASSES, SLICE], f32,
                            kind="ExternalOutput").ap()

    u_buf = nc.dram_tensor("u_buf", [N_GRAPHS, HIDDEN], f32).ap()
    m_slice = nc.dram_tensor("m_slice", [PAD_SLICE, HIDDEN], f16).ap()
    m_full = nc.dram_tensor("m_full", [N_CORES * PAD_SLICE, HIDDEN], f16,
                            addr_space="Shared").ap()

    AF = mybir.ActivationFunctionType
    OP = mybir.AluOpType

    with tile.TileContext(nc) as tc:
        with (
            tc.tile_pool(name="const", bufs=1) as cpool,
            tc.tile_pool(name="persist", bufs=1) as ppool,
            tc.tile_pool(name="work", bufs=3) as wpool,
            tc.tile_pool(name="sbig", bufs=2) as spool,
            tc.tile_pool(name="mg", bufs=16) as mgpool,
            tc.tile_pool(name="ps_a", bufs=2, space="PSUM") as ps_a,
            tc.tile_pool(name="ps_b", bufs=2, space="PSUM") as ps_b,
            tc.tile_pool(name="ps_t", bufs=2, space="PSUM") as ps_t,
            tc.tile_pool(name="ps_o", bufs=2, space="PSUM") as ps_o,
        ):
            def ctile(name, ap, shape, dt):
                t = cpool.tile(shape, dt, tag=f"c_{name}")
                nc.sync.dma_start(t[:], ap[:])
                return t

            Wg_t = ctile("Wg", Wg_d, [IN_GLOBAL, HIDDEN], f32)
            Wc1_t = ctile("Wc1", Wc1_d, [IN_LOCAL, HIDDEN], f32)
            Wc2_t = ctile("Wc2", Wc2_d, [HIDDEN, HIDDEN], f32)
            Wm_t = ctile("Wm", Wm_d, [HIDDEN, HIDDEN], f32)
            Ws_t = ctile("Ws", Ws_d, [HIDDEN, HIDDEN], f32)
            Wo_t = ctile("Wo", Wo_d, [HIDDEN, NUM_CLASSES], f32)
            bg_t = ctile("bg", bg_d, [HIDDEN, 1], f32)
            bc_t = ctile("bc", bc_d, [P, HIDDEN], f32)
            bm_t = ctile("bm", bm_d, [P, HIDDEN], f32)
            bs_t = ctile("bs", bs_d, [HIDDEN, 1], f32)
            bo_t = ctile("bo", bo_d, [NUM_CLASSES, 1], f32)
            gfT_t = ctile("gfT", gfT_d, [IN_GLOBAL, N_GRAPHS], f32)
            ubi_t = ctile("ubi", ubi_d, [P, NBLK], i32)
            iota_t = ctile("iota", iota_d, [P, k_chunks * P], f16)

            ident = cpool.tile([P, P], f32)
            make_identity(nc, ident[:])

            xT_t = ppool.tile([IN_LOCAL, PAD_SLICE], f32)
            nc.sync.dma_start(xT_t[:], xT_d[:])
            srcT_t = ppool.tile([P, nchunks], i32)
            nc.sync.dma_start(srcT_t[:], srcT_d[:])
            dstT_t = ppool.tile([P, nchunks], f16)
            nc.sync.dma_start(dstT_t[:], dstT_d[:])

            h0T_t = ppool.tile([HIDDEN, PAD_SLICE], f32)   # 6.4 MB persistent
            outT_t = ppool.tile([NUM_CLASSES, PAD_SLICE], f32)

            # ---------------- phase 0: global encoder ----------------
            for g in range(GBLK):
                gsl = slice(g * P, (g + 1) * P)
                ps1 = ps_b.tile([P, P], f32, tag="pb")
                nc.tensor.matmul(out=ps1[:], lhsT=Wg_t[:], rhs=gfT_t[:, gsl],
                                 start=True, stop=True)
                rT = wpool.tile([P, P], f32, tag="rT")
                nc.scalar.activation(out=rT[:], in_=ps1[:], func=AF.Relu,
                                     bias=bg_t[:, :1])
                ps2 = ps_b.tile([P, P], f32, tag="pb")
                nc.tensor.matmul(out=ps2[:], lhsT=Wc2_t[:], rhs=rT[:],
                                 start=True, stop=True)
                uT = wpool.tile([P, P], f32, tag="uT")
                nc.vector.tensor_copy(out=uT[:], in_=ps2[:])
                ps3 = ps_t.tile([P, P], f32, tag="pt")
                nc.tensor.transpose(out=ps3[:], in_=uT[:], identity=ident[:])
                ub = wpool.tile([P, P], f32, tag="ublk")
                nc.vector.tensor_tensor(out=ub[:], in0=ps3[:], in1=bc_t[:],
                                        op=OP.add)
                nc.sync.dma_start(u_buf[gsl, :], ub[:])

            # ---------------- phase 1: h0 / m on own slice ----------------
            for b in range(NBLK):
                bsl = slice(b * P, (b + 1) * P)
                ug = wpool.tile([P, HIDDEN], f32, tag="ug")
                nc.gpsimd.indirect_dma_start(
                    out=ug[:], out_offset=None, in_=u_buf[:],
                    in_offset=bass.IndirectOffsetOnAxis(
                        ap=ubi_t[:, b:b + 1], axis=0))
                psh = ps_b.tile([P, P], f32, tag="pb")
                nc.tensor.matmul(out=psh[:], lhsT=xT_t[:, bsl], rhs=Wc1_t[:],
                                 start=True, stop=True)
                h0 = wpool.tile([P, P], f32, tag="h0")
                nc.vector.tensor_tensor(out=h0[:], in0=psh[:], in1=ug[:],
                                        op=OP.add)
                nc.vector.tensor_scalar_max(out=h0[:], in0=h0[:], scalar1=0.0)
                pst = ps_t.tile([P, P], f32, tag="pt")
                nc.tensor.transpose(out=pst[:], in_=h0[:], identity=ident[:])
                nc.any.tensor_copy(out=h0T_t[:, bsl], in_=pst[:])
                psm = ps_b.tile([P, P], f32, tag="pb")
                nc.tensor.matmul(out=psm[:], lhsT=h0T_t[:, bsl], rhs=Wm_t[:],
                                 start=True, stop=True)
                m16 = wpool.tile([P, HIDDEN], f16, tag="m16")
                nc.vector.tensor_tensor(out=m16[:], in0=psm[:], in1=bm_t[:],
                                        op=OP.add)
                nc.vector.tensor_scalar_max(out=m16[:], in0=m16[:], scalar1=0.0)
                nc.sync.dma_start(m_slice[bsl, :], m16[:])

            # ---------------- allgather m ----------------
            nc.gpsimd.collective_compute(
                "AllGather", OP.bypass,
                replica_groups=[list(range(N_CORES))],
                ins=[m_slice[:]], outs=[m_full[:]])

            # ---------------- phase 2: scatter-add + update + readout ------
            iota_v = iota_t[:].rearrange("p (k f) -> p k f", k=k_chunks)
            for b in range(NBLK):
                bsl = slice(b * P, (b + 1) * P)
                csl = slice(b * k_chunks, (b + 1) * k_chunks)
                S = spool.tile([P, k_chunks, P], f16, tag="S")
                nc.vector.tensor_tensor(
                    out=S[:],
                    in0=dstT_t[:, csl].to_broadcast([P, k_chunks, P]),
                    in1=iota_v, op=OP.is_equal)
                pa = ps_a.tile([HIDDEN, P], f32, tag="pa")
                for k in range(k_chunks):
                    j = b * k_chunks + k
                    mg = mgpool.tile([P, HIDDEN], f16, tag="mg")
                    nc.gpsimd.indirect_dma_start(
                        out=mg[:], out_offset=None, in_=m_full[:],
                        in_offset=bass.IndirectOffsetOnAxis(
                            ap=srcT_t[:, j:j + 1], axis=0))
                    nc.tensor.matmul(out=pa[:], lhsT=mg[:], rhs=S[:, k, :],
                                     start=(k == 0), stop=False)
                nc.tensor.matmul(out=pa[:], lhsT=Ws_t[:], rhs=h0T_t[:, bsl],
                                 start=False, stop=True)
                hT = wpool.tile([HIDDEN, P], f32, tag="hT")
                nc.scalar.activation(out=hT[:], in_=pa[:], func=AF.Relu,
                                     bias=bs_t[:, :1])
                po = ps_o.tile([NUM_CLASSES, P], f32, tag="po")
                nc.tensor.matmul(out=po[:], lhsT=Wo_t[:], rhs=hT[:],
                                 start=True, stop=True)
                nc.scalar.activation(out=outT_t[:, bsl], in_=po[:],
                                     func=AF.Identity, bias=bo_t[:, :1])

            nc.sync.dma_start(outT_d[:], outT_t[:, :SLICE])

    nc.compile()
    return nc


# --------------------------------------------------------------------------
# host side
# --------------------------------------------------------------------------
def _preprocess(inputs):
    x = np.asarray(inputs["x"], dtype=np.float32)
    ei = np.asarray(inputs["edge_index"]).astype(np.int64)
    batch = np.asarray(inputs["batch"]).astype(np.int64)
    gf = np.asarray(inputs["global_feat"], dtype=np.float32)
    W = {k: np.ascontiguousarray(np.asarray(inputs[k], dtype=np.float32))
         for k in ("Wg", "bg", "Wc", "bc", "Wm", "bm", "Ws", "bs", "Wo", "bo")}

    loops = np.arange(N_NODES, dtype=np.int64)
    src_all = np.concatenate([ei[0], loops])
    dst_all = np.concatenate([ei[1], loops])
    # row of src in the AllGathered (padded) m table
    src_row = ((src_all // SLICE) * PAD_SLICE + (src_all % SLICE)).astype(np.int32)
    core_of = dst_all // SLICE

    per_core = []
    k_chunks = 1
    for c in range(N_CORES):
        sel = np.nonzero(core_of == c)[0]
        d_loc = (dst_all[sel] - c * SLICE).astype(np.int64)
        blk = d_loc // P
        order = np.argsort(blk, kind="stable")
        sel, d_loc, blk = sel[order], d_loc[order], blk[order]
        counts = np.bincount(blk, minlength=NBLK)
        k_chunks = max(k_chunks, int(-(-counts.max() // P)))
        per_core.append((sel, d_loc, blk, counts))

    cap_per_blk = k_chunks * P
    shared = {
        "gfT": np.ascontiguousarray(gf.T),
        "iota": np.tile(np.arange(P, dtype=np.float16), (P, k_chunks)),
        "Wg": W["Wg"],
        "Wc1": np.ascontiguousarray(W["Wc"][:IN_LOCAL]),
        "Wc2": np.ascontiguousarray(W["Wc"][IN_LOCAL:]),
        "Wm": W["Wm"], "Ws": W["Ws"], "Wo": W["Wo"],
        "bg_c": W["bg"].reshape(HIDDEN, 1),
        "bc_b": np.tile(W["bc"], (P, 1)),
        "bm_b": np.tile(W["bm"], (P, 1)),
        "bs_c": W["bs"].reshape(HIDDEN, 1),
        "bo_c": W["bo"].reshape(NUM_CLASSES, 1),
    }

    in_maps = []
    for c in range(N_CORES):
        sel, d_loc, blk, counts = per_core[c]
        cum = np.cumsum(counts) - counts
        within = np.arange(len(sel)) - np.repeat(cum, counts)
        pos = blk * cap_per_blk + within
        src_pad = np.zeros(NBLK * cap_per_blk, np.int32)
        dst_pad = np.full(NBLK * cap_per_blk, -1.0, np.float16)
        src_pad[pos] = src_row[sel]
        dst_pad[pos] = (d_loc % P).astype(np.float16)

        bpad = np.zeros(PAD_SLICE, np.int32)
        bpad[:SLICE] = batch[c * SLICE:(c + 1) * SLICE]
        xT = np.zeros((IN_LOCAL, PAD_SLICE), np.float32)
        xT[:, :SLICE] = x[c * SLICE:(c + 1) * SLICE].T

        m = dict(shared)
        m.update({
            "xT": xT,
            "ubi": np.ascontiguousarray(bpad.reshape(NBLK, P).T),
            "srcT": np.ascontiguousarray(src_pad.reshape(-1, P).T),
            "dstT": np.ascontiguousarray(dst_pad.reshape(-1, P).T),
        })
        in_maps.append(m)
    return k_chunks, in_maps


# --------------------------------------------------------------------------
# profiling hook (NTFF via the axon PJRT .so; absent module in this image)
# --------------------------------------------------------------------------
def _profile_hook():
    so = "/opt/axon/libaxon_pjrt.so"
    if not os.path.exists(so):
        return None
    lib = ctypes.CDLL(so)
    if not hasattr(lib, "axon_start_nrt_profile"):
        return None
    lib.axon_start_nrt_profile.argtypes = [ctypes.POINTER(ctypes.c_int64),
                                           ctypes.c_size_t]
    lib.axon_start_nrt_profile.restype = ctypes.c_int64
    lib.axon_stop_nrt_profile.argtypes = [ctypes.c_char_p]
    lib.axon_stop_nrt_profile.restype = ctypes.c_int64

    @contextlib.contextmanager
    def hook(output_dir, device_ids):
        import jax
        jax.devices()
        if device_ids:
            ids = (ctypes.c_int64 * len(device_ids))(*device_ids)
            rc = lib.axon_start_nrt_profile(ids, len(device_ids))
        else:
            rc = lib.axon_start_nrt_profile(None, 0)
        if rc != 0:
            raise RuntimeError(f"axon_start_nrt_profile rc={rc}")
        try:
            yield
        finally:
            n = lib.axon_stop_nrt_profile(str(output_dir).encode())
            print(f"profile: {n} file(s) written to {output_dir}",
                  file=sys.stderr)

    return hook


def _run(nc, in_maps):
    from concourse import bass2jax
    trace_dir = os.environ.get("GNN_TRACE_DIR", "")
    if not trace_dir:
        return bass2jax.run_bass_via_pjrt(nc, in_maps, n_cores=N_CORES)
    hook = _profile_hook()
    if hook is None:
        return bass2jax.run_bass_via_pjrt(nc, in_maps, n_cores=N_CORES)
    os.makedirs(trace_dir, exist_ok=True)
    trace_cores = [int(t) for t in
                   os.environ.get("GNN_TRACE_CORES", "0").split(",")]
    with hook(trace_dir, trace_cores):
        results = bass2jax.run_bass_via_pjrt(nc, in_maps, n_cores=N_CORES)
    try:
        from concourse._compat import FishPath
        import gauge.profiler as gprof
        profile = gprof.Profile(
            profile_path=FishPath(trace_dir), kernel_dev_mode=True,
            profile_on_exit=False, bass_kernel=nc.m,
            offline_processing=True, fname="*_body*")
        profile.convert_ntffs_to_json(tuple(trace_cores))
        j = profile.load_json(trace_cores[0])
        last_run["summary"] = j["summary"][0] if j else None
        last_run["exec_time_ns"] = (
            int(j["summary"][0]["total_time"] * 1000) if j else None)
        last_run["profile_json"] = str(profile.json_path(trace_cores[0]))
        last_run["profile_obj"] = profile
    except Exception as e:  # profiling must never break the run
        print(f"profile post-processing failed: {e}", file=sys.stderr)
    return results


def kernel(**inputs) -> np.ndarray:
    k_chunks, in_maps = _preprocess(inputs)
    nc = _prog_cache.get(k_chunks)
    if nc is None:
        nc = _build(k_chunks)
        _prog_cache[k_chunks] = nc
    last_run.clear()
    last_run["k_chunks"] = k_chunks
    results = _run(nc, in_maps)
    outT = np.concatenate([results[c]["outT"] for c in range(N_CORES)], axis=1)
    return np.ascontiguousarray(outT.T.astype(np.float32))
